# revision 39
# baseline (speedup 1.0000x reference)
"""Sharded 2-layer GCN (PyG GCNConv x2 + ReLU) on 8 trn2 NeuronCores.

Self-contained: host-side graph preprocessing (sharding/padding/index build),
two SPMD Bass programs (launch 1: aggregate z -> x1 -> y; launch 2: aggregate
y -> output), host relay of the y-table between launches.
"""
"""Host-side graph preprocessing for the sharded GCN kernel.

Shards: core k owns dst nodes [k*6250, (k+1)*6250). Gather tables are the
node-feature matrix split in two halves of 25000 rows (+1 zero row each)
because dma_gather indices are int16.

Per (node, half): the list of edge sources, padded to a multiple of 4 with
ZERO row indices -> one "run" of length g (the degree class). Runs of each
class are round-robined across the 128 partitions; per (half, class) the
per-partition run count K is padded (with all-ZERO runs) to a value uniform
across partitions AND across cores (SPMD: one program for all cores).

Gather position i (within a half's region) lands at gbuf[i%128, i//128, :].
A run = g consecutive columns of one partition. Output slot (p, s) of the
half's O buffer holds the run's sum; it is dinv-scaled then scatter-added
into agg_hbm[node_local] (node_local = canonical row). Slot padding scatters
into a dump row.
"""
import numpy as np

N_NODES = 50000
N_CORES = 8
PER_CORE = N_NODES // N_CORES          # 6250
HALF = N_NODES // 2                    # 25000
ZERO_LOCAL = HALF                      # zero row local index in each half-table
N_CANON = 6400                         # 128 * 50 canonical rows per core
CANON_COLS = N_CANON // 128            # 50
DUMP_ROW = N_CANON                     # scatter dump row (agg_hbm has N_CANON+1 rows)
TILE_COLS = 48                         # gather call max columns (gbuf tile cols)
P = 128
NQ = 4                                 # SWDGE queues (gathers+scatters fan out)


def wrap_idx16(idx):
    """[n] -> [128, ceil(n/16)] int16: position i at [i%16, i//16], 16-row
    block replicated across all 128 partitions."""
    idx = np.asarray(idx)
    n = len(idx)
    cols = -(-n // 16)
    t = np.zeros((16, cols), dtype=np.int16)
    t[np.arange(n) % 16, np.arange(n) // 16] = idx.astype(np.int16)
    return np.tile(t, (8, 1))


def build_graph_structs(edge_index):
    """edge_index: [2, E] int array (no self loops; we add them).
    Returns (sched, per_core) where sched is the SPMD-uniform schedule and
    per_core is a list of dicts of per-core input arrays."""
    src = np.asarray(edge_index[0], dtype=np.int64)
    dst = np.asarray(edge_index[1], dtype=np.int64)

    # self-loops are NOT gathered: their contribution dinv[n]^2 * x[n] is a
    # contiguous read, folded into the per-rep agg init image on the host.
    # deg still counts the self-loop (reference semantics).
    deg = np.bincount(dst, minlength=N_NODES).astype(np.float64) + 1.0
    dinv = np.where(deg > 0, 1.0 / np.sqrt(deg), 0.0).astype(np.float32)

    # bucket edges by dst: order[] gives edge ids sorted by dst
    order = np.argsort(dst, kind="stable")
    s_sorted = src[order]
    starts = np.searchsorted(dst[order], np.arange(N_NODES + 1))

    # ---- pass 1: per-core runs grouped by (half, natural class g=2*ceil(r/2)) ----
    # runs_by[core][half][g] = list of (node_local, srcs_local_unpadded)
    runs_by = [[{}, {}] for _ in range(N_CORES)]
    for core in range(N_CORES):
        lo = core * PER_CORE
        for n in range(lo, lo + PER_CORE):
            ss = s_sorted[starts[n]:starts[n + 1]]
            nl = n - lo
            for h in (0, 1):
                sh = ss[ss < HALF] if h == 0 else ss[ss >= HALF] - HALF
                r = len(sh)
                if r == 0:
                    continue
                g = r  # exact-degree classes: no even-rounding zero-pad
                assert g <= TILE_COLS, f"run too long: {g}"
                runs_by[core][h].setdefault(g, []).append((nl, sh))

    # ---- uniform schedule with promotion: per class take K = min over cores
    # of floor(avail/128); excess runs get promoted into the next class
    # (padded further). Only the largest class pays ceil-rounding. ----
    sched = {"halves": []}
    assign_by = [[{}, {}] for _ in range(N_CORES)]  # [core][h][g] -> runs assigned
    for h in (0, 1):
        classes = sorted(set().union(*[set(runs_by[c][h].keys()) for c in range(N_CORES)]))
        carry = [[] for _ in range(N_CORES)]
        cls = []
        for gi, g in enumerate(classes):
            avail = [runs_by[c][h].get(g, []) + carry[c] for c in range(N_CORES)]
            last = gi == len(classes) - 1
            if last:
                K = max(-(-len(a) // P) for a in avail)
            else:
                K = min(len(a) // P for a in avail)
            for c in range(N_CORES):
                assign_by[c][h][g] = avail[c][:K * P]
                carry[c] = avail[c][K * P:]
                assert not (last and carry[c])
            if K > 0:
                cls.append((g, K))
        # column/slot bases
        out_cls = []
        col = 0
        slot = 0
        for g, K in cls:
            out_cls.append({"g": g, "K": K, "col0": col, "slot0": slot})
            col += K * g
            slot += K
        # gather calls: chunks of <= TILE_COLS columns, boundaries at run
        # boundaries (runs are g-periodic within each class region)
        calls = []
        cur0 = 0
        while cur0 < col:
            # find the largest boundary <= cur0 + TILE_COLS
            end = cur0
            segs = []  # (class_idx, runs, col_start) intersections
            for ci, c in enumerate(out_cls):
                g, K = c["g"], c["K"]
                c_end = c["col0"] + K * g
                if c_end <= cur0 or c["col0"] >= cur0 + TILE_COLS:
                    continue
                a = max(cur0, c["col0"])
                avail_runs = (min(c_end, cur0 + TILE_COLS) - a) // g
                if avail_runs == 0:
                    break  # run doesn't fit in the remaining tile space
                b = a + avail_runs * g
                segs.append({"cls": ci, "col0": a, "runs": avail_runs,
                             "slot0": c["slot0"] + (a - c["col0"]) // g})
                end = b
                if b < c_end:
                    break  # tile full inside this class
            assert end > cur0, (cur0, out_cls)
            calls.append({"col0": cur0, "cols": end - cur0, "segs": segs})
            cur0 = end
        sched["halves"].append({
            "classes": out_cls, "cols": col, "slots": slot, "calls": calls})

    # ---- pass 2: per-core arrays ----
    per_core = []
    for core in range(N_CORES):
        data = {}
        for h in (0, 1):
            hs = sched["halves"][h]
            C, S = hs["cols"], hs["slots"]
            gidx = np.full(C * P, ZERO_LOCAL, dtype=np.int64)
            sctr = np.full(S * P, DUMP_ROW, dtype=np.int64)
            dinv_slot = np.zeros((P, S), dtype=np.float32)
            for c in hs["classes"]:
                g, K = c["g"], c["K"]
                runs = assign_by[core][h].get(g, [])
                for i, (nl, srcs) in enumerate(runs):
                    p, k = i % P, i // P
                    assert k < K
                    col = c["col0"] + k * g
                    gidx[(col + np.arange(len(srcs))) * P + p] = srcs
                    s = c["slot0"] + k
                    sctr[s * P + p] = nl
                    dinv_slot[p, s] = dinv[core * PER_CORE + nl]
            data[f"gidx{h}"] = wrap_idx16(gidx)
            data[f"sctr{h}"] = wrap_idx16(sctr)
            data[f"dinv{h}"] = dinv_slot
        # canonical dinv tile [128, 50]: node n=p*CANON_COLS+c -> [p, c]
        # (matches the contiguous aggM reload layout)
        dc = np.zeros((P, CANON_COLS), dtype=np.float32)
        nn = np.arange(PER_CORE)
        dc[nn // CANON_COLS, nn % CANON_COLS] = dinv[core * PER_CORE + nn]
        data["dinv_canon"] = dc
        per_core.append(data)

    return sched, per_core, dinv


def make_tables(x, dinv):
    """x: [N_NODES, F] -> gather table [2*(HALF+1), F] f32 (prescaled by dinv)."""
    xs = (x * dinv[:, None]).astype(np.float32)
    F = x.shape[1]
    z = np.zeros((1, F), dtype=np.float32)
    return np.concatenate([xs[:HALF], z, xs[HALF:], z], axis=0)




# ================= Bass program builders =================
import concourse.bacc as bacc
import concourse.bass as bass
import concourse.mybir as mybir
from concourse.library_config import mlp
from concourse._compat import cdiv


F = 64          # feature width of gather tables
TBL = 2 * (HALF + 1)
GBUF_COLS = TILE_COLS  # gather tile columns


def build(sched, mode, reps=1):
    assert mode in ("l1", "l2")
    nc = bacc.Bacc("TRN2", num_swdge_queues=NQ)
    dt = mybir.dt.float32
    NBUF = 6  # gather buffers (l1 is SBUF-tight; 8 measured no better for l2)

    halves = sched["halves"]
    calls = []  # flattened (h, call)
    for h in (0, 1):
        for c in halves[h]["calls"]:
            calls.append((h, c))
    NCALL = len(calls)
    queue_of = [i % NQ for i in range(NCALL)]  # round-robin across SWDGE queues
    cum_r = [0]
    for h, c in calls:
        cum_r.append(cum_r[-1] + len(c["segs"]))
    RPC = cum_r[-1]           # reduces per rep
    S = [halves[0]["slots"], halves[1]["slots"]]
    C = [halves[0]["cols"], halves[1]["cols"]]
    NMM = cdiv(N_CANON, 512)  # 13
    SCT = 16 * NQ             # s_sct increments per half (NQ split scatters)
    # per-half scatter slot splits (disjoint node rows -> safe concurrently;
    # dump-row RMW races only clobber garbage)
    ssplit = [[(S[h] * qi) // NQ for qi in range(NQ + 1)] for h in (0, 1)]

    # ---------------- DRAM tensors ----------------
    table = nc.dram_tensor("table", [TBL, F], dt, kind="ExternalInput")
    gidx = [nc.dram_tensor(f"gidx{h}", [P, C[h] * 8], mybir.dt.int16, kind="ExternalInput")
            for h in (0, 1)]
    sctr = [nc.dram_tensor(f"sctr{h}", [P, cdiv(S[h] * P, 16)], mybir.dt.int16, kind="ExternalInput")
            for h in (0, 1)]
    dinv = [nc.dram_tensor(f"dinv{h}", [P, S[h]], dt, kind="ExternalInput") for h in (0, 1)]
    # double-buffered HBM agg scratch: rep r scatters into aggs[r%2], so the
    # next rep's scatters don't wait on this rep's reload
    aggs = [nc.dram_tensor("agg_hbm", [N_CANON + 1, F], dt, kind="ExternalOutput"),
            nc.dram_tensor("agg_hbm2", [N_CANON + 1, F], dt, kind="ExternalOutput")]
    if mode == "l1":
        dinv_canon = nc.dram_tensor("dinv_canon", [P, CANON_COLS], dt, kind="ExternalInput")
        W1 = nc.dram_tensor("W1", [64, 128], dt, kind="ExternalInput")
        W2 = nc.dram_tensor("W2", [128, 64], dt, kind="ExternalInput")
        b1 = nc.dram_tensor("b1", [128, 1], dt, kind="ExternalInput")
        ident = nc.dram_tensor("ident", [128, 128], dt, kind="ExternalInput")
        y_hbm = nc.dram_tensor("y_hbm", [N_CANON, F], dt, kind="ExternalOutput")
    else:
        out_hbm = nc.dram_tensor("out_hbm", [N_CANON, F], dt, kind="ExternalOutput")
    # per-rep agg init image (host-computed): self-loop term dinv^2*x, plus b2
    # for l2. HBM->HBM copy, no SBUF staging.
    initv = nc.dram_tensor("initv", [N_CANON, F], dt, kind="ExternalInput")

    NLOAD = 4 + 2 + (5 if mode == "l1" else 0)

    import contextlib
    with contextlib.ExitStack() as ctx:
        def sb(name, shape, d=dt):
            return ctx.enter_context(nc.sbuf_tensor(name, shape, d))

        def ps(name, shape):
            return ctx.enter_context(nc.psum_tensor(name, shape, dt))

        def sem(name):
            return ctx.enter_context(nc.semaphore(name))

        gbuf = [sb(f"gbuf{i}", [P, GBUF_COLS, F]) for i in range(NBUF)]
        gidx_sb = [sb(f"gidx_sb{h}", [P, C[h] * 8], mybir.dt.int16) for h in (0, 1)]
        sctr_sb = [sb(f"sctr_sb{h}", [P, cdiv(S[h] * P, 16)], mybir.dt.int16) for h in (0, 1)]
        dinv_sb = [sb(f"dinv_sb{h}", [P, S[h]]) for h in (0, 1)]
        O = [sb(f"O{h}", [P, S[h], F]) for h in (0, 1)]
        # l2 double-buffers aggM/outb so reload(r) need not wait relu(r-1)
        aggMs = [sb("aggM", [P, CANON_COLS, F])] + \
            ([sb("aggM2", [P, CANON_COLS, F])] if mode == "l2" else [])
        aggM = aggMs[0]
        if mode == "l1":
            dinvc_sb = sb("dinvc_sb", [P, CANON_COLS])
            W1_sb = sb("W1_sb", [64, 128])
            W2_sb = sb("W2_sb", [128, 64])
            b1_sb = sb("b1_sb", [128, 1])
            ident_sb = sb("ident_sb", [128, 128])
            aggT = sb("aggT", [64, N_CANON])
            x1T = sb("x1T", [128, N_CANON])
            yT_sb = sb("yT_sb", [64, N_CANON])
            ybuf = [sb(f"ybuf{i}", [P, F]) for i in range(2)]
            tp_ps = [ps(f"tp_ps{i}", [64, 128]) for i in range(2)]
            x1_ps = [ps(f"x1_ps{i}", [128, 512]) for i in range(2)]
            yT_ps = [ps(f"yT_ps{i}", [64, 512]) for i in range(2)]
            yb_ps = [ps(f"yb_ps{i}", [128, 64]) for i in range(2)]
        else:
            outbs = [sb("outb", [P, CANON_COLS * F]), sb("outb2", [P, CANON_COLS * F])]

        s_ld = sem("s_ld")
        s_g = [sem(f"s_g{i}") for i in range(NBUF)]
        s_r = sem("s_r")
        s_sc = sem("s_sc")
        s_sct = sem("s_sct")
        s_mrg = sem("s_mrg")
        if mode == "l1":
            s_t = sem("s_t")
            s_e = sem("s_e")
            s_m1 = sem("s_m1")
            s_a1 = sem("s_a1")
            s_m2 = sem("s_m2")
            s_d2 = sem("s_d2")
            s_t2 = sem("s_t2")
            s_d3 = sem("s_d3")
            s_w = [sem("s_w0"), sem("s_w1")]
        else:
            s_o = sem("s_o")
            s_w = sem("s_w")
        s_b2 = sem("s_b2")

        with nc.Block() as block:

            @block.sync
            def _(sync):
                for h in (0, 1):
                    sync.dma_start(gidx_sb[h][:], gidx[h][:]).then_inc(s_ld, 16)
                    sync.dma_start(sctr_sb[h][:], sctr[h][:]).then_inc(s_ld, 16)
                    sync.dma_start(dinv_sb[h][:], dinv[h][:]).then_inc(s_ld, 16)
                if mode == "l1":
                    sync.dma_start(dinvc_sb[:], dinv_canon[:]).then_inc(s_ld, 16)
                    sync.dma_start(W1_sb[:], W1[:]).then_inc(s_ld, 16)
                    sync.dma_start(W2_sb[:], W2[:]).then_inc(s_ld, 16)
                    sync.dma_start(b1_sb[:], b1[:]).then_inc(s_ld, 16)
                    sync.dma_start(ident_sb[:], ident[:]).then_inc(s_ld, 16)
                # init both agg buffers (for reps 0 and 1) with the host image
                for j in (0, 1):
                    sync.dma_start(aggs[j][:N_CANON, :], initv[:]).then_inc(s_b2, 16)

                for rep in range(reps):
                    sync.wait_ge(s_sct, 2 * SCT * (rep + 1))
                    if mode == "l1" and rep >= 1:
                        sync.wait_ge(s_t, CANON_COLS * rep)   # aggM WAR
                    if mode == "l2" and rep >= 2:
                        sync.wait_ge(s_o, rep - 1)            # aggMs[rep%2] WAR
                    # contiguous per-partition layout (128 big descriptors, not
                    # 6400 small ones); aggM[p, c] = node p*CANON_COLS+c.
                    # l1's transpose pipeline then emits y in (c, p) tile order
                    # which the host unscrambles for free.
                    sync.dma_start(
                        aggMs[rep % len(aggMs)][:],
                        aggs[rep % 2][:N_CANON, :].rearrange("(p c) f -> p c f", c=CANON_COLS)
                    ).then_inc(s_mrg, 16)
                    if rep + 2 < reps:
                        # re-init this buffer for rep+2 (FIFO after the reload
                        # read on the same engine ring)
                        sync.dma_start(
                            aggs[rep % 2][:N_CANON, :], initv[:]
                        ).then_inc(s_b2, 16)

                    if mode == "l1":
                        for c in range(CANON_COLS):
                            gct = rep * CANON_COLS + c
                            sync.wait_ge(s_d3, gct + 1)
                            sync.dma_start(
                                y_hbm[c * P:(c + 1) * P, :], ybuf[gct % 2][:]
                            ).then_inc(s_w[gct % 2], 16)
                    else:
                        sync.wait_ge(s_o, rep + 1)
                        sync.dma_start(
                            out_hbm[:].rearrange("(p c) f -> p c f", c=CANON_COLS),
                            outbs[rep % 2][:].rearrange("p (c f) -> p c f", f=F),
                        ).then_inc(s_w, 16)
                if mode == "l1":
                    tot = reps * CANON_COLS
                    sync.wait_ge(s_w[0], 16 * cdiv(tot, 2))
                    sync.wait_ge(s_w[1], 16 * (tot // 2))
                else:
                    sync.wait_ge(s_w, 16 * reps)

            @block.gpsimd
            def _(gpsimd):
                gpsimd.load_library(mlp)
                gpsimd.wait_ge(s_ld, 16 * NLOAD)
                tbl_ap = [table[0:HALF + 1, :], table[HALF + 1:TBL, :]]
                for rep in range(reps):
                    for i, (h, c) in enumerate(calls):
                        gi = rep * NCALL + i
                        b = gi % NBUF
                        if gi >= NBUF:
                            # reduces of call gi-NBUF done -> gbuf slot free
                            repp, ip = divmod(gi - NBUF, NCALL)
                            gpsimd.wait_ge(s_r, repp * RPC + cum_r[ip + 1])
                            gpsimd.wait_ge(s_g[b], 16 * (gi // NBUF))
                        cols = c["cols"]
                        n_idx = cols * P
                        pos0 = c["col0"] * P
                        gpsimd.dma_gather(
                            gbuf[b][:, 0:cols, :],
                            tbl_ap[h],
                            gidx_sb[h][:, pos0 // 16:(pos0 + n_idx) // 16],
                            n_idx,
                            n_idx,
                            F,
                            single_packet=False,
                            queue_num=queue_of[i],
                        ).then_inc(s_g[b], 16)
                    for h in (0, 1):
                        gpsimd.wait_ge(s_sc, rep * 2 + h + 1)
                        gpsimd.wait_ge(s_b2, 16 * (rep + 1))
                        if rep >= 1 or h == 1:
                            # serialize the two halves (RMW on same agg rows)
                            gpsimd.wait_ge(s_sct, 2 * SCT * rep + SCT * h)
                        if rep >= 2 and h == 0:
                            # aggs[rep%2] reused; reload of rep-2 must be done
                            gpsimd.wait_ge(s_mrg, 16 * (rep - 1))
                        for qi in range(NQ):
                            a, bnd = ssplit[h][qi], ssplit[h][qi + 1]
                            gpsimd.dma_scatter_add(
                                aggs[rep % 2][:, :],
                                O[h][:, a:bnd, :],
                                sctr_sb[h][:, a * P // 16:bnd * P // 16],
                                (bnd - a) * P,
                                (bnd - a) * P,
                                F,
                                single_packet=False,
                                queue_num=qi,
                            ).then_inc(s_sct, 16)
                gpsimd.wait_ge(s_sct, 2 * SCT * reps)

            @block.vector
            def _(vector):
                for rep in range(reps):
                    r = rep * RPC
                    for i, (h, c) in enumerate(calls):
                        gi = rep * NCALL + i
                        vector.wait_ge(s_g[gi % NBUF], 16 * (gi // NBUF + 1))
                        if rep >= 1 and (i == 0 or calls[i - 1][0] != h):
                            # O[h] WAR vs rep-1's scatter read
                            vector.wait_ge(s_sct, 2 * SCT * (rep - 1) + SCT * (h + 1))
                        for seg in c["segs"]:
                            g = halves[h]["classes"][seg["cls"]]["g"]
                            a = seg["col0"] - c["col0"]
                            nr = seg["runs"]
                            src = gbuf[gi % NBUF][:, a:a + nr * g, :].rearrange(
                                "p (k g) f -> p k f g", g=g)
                            vector.tensor_reduce(
                                out=O[h][:, seg["slot0"]:seg["slot0"] + nr, :],
                                in_=src,
                                axis=mybir.AxisListType.X,
                                op=mybir.AluOpType.add,
                            ).then_inc(s_r, 1)
                            r += 1
                        if i == NCALL - 1 or calls[i + 1][0] != h:
                            # last call of half h -> dinv scale the whole O[h]
                            # (explicit self-wait: DVE ops pipeline, RAW needs sync)
                            vector.wait_ge(s_r, r)
                            vector.tensor_tensor(
                                out=O[h][:],
                                in0=O[h][:],
                                in1=dinv_sb[h][:].to_broadcast([P, S[h], F]),
                                op=mybir.AluOpType.mult,
                            ).then_inc(s_sc, 1)
                    # d2/d3 (l1) and relu (l2) live on the ACT engine: the DVE
                    # stream must stay pure reduces+scales, or those ops' waits
                    # on the MLP/reload chain would block the next rep's
                    # reduces (DVE is in-order) and stall the gather pipeline.

            if mode == "l1":

                @block.tensor
                def _(tensor):
                    for rep in range(reps):
                        tensor.wait_ge(s_mrg, 16 * (rep + 1))
                        for c in range(CANON_COLS):
                            gct = rep * CANON_COLS + c
                            if gct >= 2:
                                tensor.wait_ge(s_e, gct - 1)
                            tensor.transpose(
                                out=tp_ps[gct % 2][:],
                                in_=aggM[:, c, :],
                                identity=ident_sb[:],
                            ).then_inc(s_t, 1)
                        for j in range(NMM):
                            gj = rep * NMM + j
                            nj = min(512, N_CANON - j * 512)
                            tensor.wait_ge(
                                s_e, rep * CANON_COLS + min(CANON_COLS, cdiv(j * 512 + nj, 128)))
                            if gj >= 2:
                                tensor.wait_ge(s_a1, gj - 1)
                            tensor.matmul(
                                out=x1_ps[gj % 2][:, 0:nj],
                                lhsT=W1_sb[:],
                                rhs=aggT[:, j * 512:j * 512 + nj],
                                start=True,
                                stop=True,
                            ).then_inc(s_m1, 1)
                        for j in range(NMM):
                            gj = rep * NMM + j
                            nj = min(512, N_CANON - j * 512)
                            tensor.wait_ge(s_a1, gj + 1)
                            if gj >= 2:
                                tensor.wait_ge(s_d2, gj - 1)
                            tensor.matmul(
                                out=yT_ps[gj % 2][:, 0:nj],
                                lhsT=W2_sb[:],
                                rhs=x1T[:, j * 512:j * 512 + nj],
                                start=True,
                                stop=True,
                            ).then_inc(s_m2, 1)
                        for c in range(CANON_COLS):
                            gct = rep * CANON_COLS + c
                            tensor.wait_ge(s_d2, rep * NMM + cdiv((c + 1) * 128, 512))
                            if gct >= 2:
                                tensor.wait_ge(s_d3, gct - 1)
                            tensor.transpose(
                                out=yb_ps[gct % 2][:],
                                in_=yT_sb[:, c * P:(c + 1) * P],
                                identity=ident_sb[0:64, 0:64],
                            ).then_inc(s_t2, 1)

                @block.scalar
                def _(scalar):
                    for rep in range(reps):
                        for c in range(CANON_COLS):
                            gct = rep * CANON_COLS + c
                            scalar.wait_ge(s_t, gct + 1)
                            scalar.copy(
                                out=aggT[:, c * P:(c + 1) * P],
                                in_=tp_ps[gct % 2][:],
                            ).then_inc(s_e, 1)
                        for j in range(NMM):
                            gj = rep * NMM + j
                            nj = min(512, N_CANON - j * 512)
                            scalar.wait_ge(s_m1, gj + 1)
                            scalar.activation(
                                out=x1T[:, j * 512:j * 512 + nj],
                                in_=x1_ps[gj % 2][:, 0:nj],
                                func=mybir.ActivationFunctionType.Relu,
                                bias=b1_sb[:, 0:1],
                            ).then_inc(s_a1, 1)
                        for j in range(NMM):
                            gj = rep * NMM + j
                            nj = min(512, N_CANON - j * 512)
                            scalar.wait_ge(s_m2, gj + 1)
                            scalar.copy(
                                out=yT_sb[:, j * 512:j * 512 + nj],
                                in_=yT_ps[gj % 2][:, 0:nj],
                            ).then_inc(s_d2, 1)
                        for c in range(CANON_COLS):
                            gct = rep * CANON_COLS + c
                            scalar.wait_ge(s_t2, gct + 1)
                            if gct >= 2:
                                scalar.wait_ge(s_w[gct % 2], 16 * (gct // 2))
                            scalar.activation(
                                out=ybuf[gct % 2][:],
                                in_=yb_ps[gct % 2][:],
                                func=mybir.ActivationFunctionType.Copy,
                                scale=dinvc_sb[:, c:c + 1],
                            ).then_inc(s_d3, 1)

            else:

                @block.scalar
                def _(scalar):
                    for rep in range(reps):
                        scalar.wait_ge(s_mrg, 16 * (rep + 1))
                        if rep >= 2:
                            scalar.wait_ge(s_w, 16 * (rep - 1))  # outbs[rep%2] WAR
                        scalar.activation(
                            out=outbs[rep % 2][:],
                            in_=aggMs[rep % 2][:].rearrange("p c f -> p (c f)"),
                            func=mybir.ActivationFunctionType.Relu,
                        ).then_inc(s_o, 1)

    nc.compile()
    return nc


# ================= host-side input assembly + kernel entry =================
import concourse.bass_utils as _bass_utils


def _initv_slices(base):
    """Pad per-core [PER_CORE, F] rows to [N_CANON, F] init images."""
    out = []
    for core in range(N_CORES):
        iv = np.zeros((N_CANON, F), dtype=np.float32)
        iv[:PER_CORE] = base[core * PER_CORE:(core + 1) * PER_CORE]
        out.append(iv)
    return out


def _core_inputs_l1(pc, table, z32, dinv, W1np, b1np, W2np):
    ident = np.eye(128, dtype=np.float32)
    initvs = _initv_slices(dinv[:, None] ** 2 * z32)  # self-loop term
    ins = []
    for core in range(N_CORES):
        d = pc[core]
        ins.append({
            "table": table,
            "gidx0": d["gidx0"], "gidx1": d["gidx1"],
            "sctr0": d["sctr0"], "sctr1": d["sctr1"],
            "dinv0": d["dinv0"], "dinv1": d["dinv1"],
            "dinv_canon": d["dinv_canon"],
            "initv": initvs[core],
            "W1": np.asarray(W1np, dtype=np.float32),
            "W2": np.asarray(W2np, dtype=np.float32),
            "b1": np.asarray(b1np, dtype=np.float32).reshape(128, 1),
            "ident": ident,
        })
    return ins


def _core_inputs_l2(pc, ys, b2np, dinv):
    """ys: node-major relayed y (= dinv * x1@W2). Self term = dinv[n]*ys[n]."""
    b2v = np.asarray(b2np, dtype=np.float32).reshape(1, F)
    initvs = _initv_slices(b2v + dinv[:, None] * ys)
    ys_table = _raw_table(ys)
    ins = []
    for core in range(N_CORES):
        d = pc[core]
        ins.append({
            "table": ys_table,
            "gidx0": d["gidx0"], "gidx1": d["gidx1"],
            "sctr0": d["sctr0"], "sctr1": d["sctr1"],
            "dinv0": d["dinv0"], "dinv1": d["dinv1"],
            "initv": initvs[core],
        })
    return ins


def _raw_table(xs):
    z = np.zeros((1, xs.shape[1]), dtype=np.float32)
    return np.concatenate([xs[:HALF], z, xs[HALF:], z], axis=0).astype(np.float32)


def _y_to_nodes(y_hbm):
    """y_hbm row c*128+p holds node p*CANON_COLS+c; return node-major rows."""
    yh = np.asarray(y_hbm).reshape(CANON_COLS, P, -1)
    return yh.transpose(1, 0, 2).reshape(N_CANON, -1)[:PER_CORE]


_CACHE = {}


def _get_programs(edge_index):
    key = hash(edge_index.tobytes())
    if key not in _CACHE:
        sched, pc, dinv = build_graph_structs(edge_index)
        _CACHE[key] = (sched, pc, dinv, build(sched, "l1"), build(sched, "l2"))
    return _CACHE[key]


def kernel(z, edge_index, W1, b1, W2, b2):
    z = np.asarray(z)
    edge_index = np.asarray(edge_index)
    sched, pc, dinv, nc1, nc2 = _get_programs(edge_index)

    z32 = np.asarray(z, dtype=np.float32)
    table1 = make_tables(z32, dinv)
    ins1 = _core_inputs_l1(pc, table1, z32, dinv, W1, b1, W2)
    r1 = _bass_utils.run_bass_kernel_spmd(nc1, ins1, core_ids=list(range(8)))
    ys = np.concatenate(
        [_y_to_nodes(r1.results[c]["y_hbm"]) for c in range(N_CORES)], axis=0)

    ins2 = _core_inputs_l2(pc, ys, b2, dinv)
    r2 = _bass_utils.run_bass_kernel_spmd(nc2, ins2, core_ids=list(range(8)))
    out = np.concatenate(
        [r2.results[c]["out_hbm"][:PER_CORE] for c in range(N_CORES)], axis=0)
    return out.astype(np.float32)

